# revision 81
# baseline (speedup 1.0000x reference)
"""DenseGINEConv on 8 TRN2 NeuronCores (Bass/Tile).

Reference computation (B=4, N=512, F=64, H=128):
    msg  = leaky_relu(adj[b,i,j] * (x[b,i,f] + edge_attr[b,i,j,f]), 0.01)
    agg  = sum_i msg                         # (B, N, F) indexed by destination j
    out  = x + agg
    h    = leaky_relu(out @ W1 + b1) @ W2 + b2
    res  = where(mask[b,j], h, 0)

Key facts used:
  * adj >= 0 (uniform fill), so leaky_relu(adj*z) = adj * leaky_relu(z).
  * lrelu(z) = 0.01*z + 0.99*relu(z): the linear part needs NO activation
    pass at all (it is a matmul on raw z against a 0.01-scaled adjacency),
    and relu is cheaper than lrelu on DVE (tensor_tensor max vs a zero
    broadcast runs in 2x mode, 0.52ns/elem).
  * Rows with mask=0 produce zero output, so each core only processes its
    compacted list of kept destination nodes (host-side j-compaction).
  * Everything is transported in bf16: edge_attr dominates HBM traffic and
    the DMA engines are the binding resource (360 GB/s aggregate), so
    halving the bytes halves the stream time. The extra rounding error is
    ~5e-3 relative, well inside the 2e-2 gate.

Per-core pipeline, j-blocks tapering 24 -> 12 -> 6 (software-pipelined with
LOOKAHEAD=3 so prefills/DMAs run blocks ahead of consumers):
  1. Block 0 goes over HWDGE plain (no prefill dependency, stream starts
     ~2.6us in) with an explicit DVE x-add. Every other block: the z tile
     [128, 4*JW*64] bf16 is pre-filled with broadcast x by a DVE copy (4x
     mode), then ONE SWDGE DMA per block streams the edge slab for all
     four i-blocks on top with the DMA engines' inline CCE adder
     (accum_op=add): the elementwise x+e add costs no engine time.
  2. u = relu(z), jw-split between ScalarE (0.833ns/elem, act cost is
     dtype-independent) and DVE; early blocks lean ACT, late blocks DVE
     (whose prefill duty is over by then) so the last relus are data-paced.
  3. Aggregation: for each destination node j, 8 single-column matmuls
     (z-slab and u-slab per i-block as stationary weights, adjacency
     columns 0.01*adj / 0.99*adj as the moving operand) accumulate
     oT[:, j] = sum_i adj[i,j]*lrelu(...) directly into one [F, Jp] PSUM
     tile. No cross products, diagonal masks, reductions, or transposes.
     The raw-z matmuls need only the DMA, so they overlap the relu. Only
     the 0.99 copy is shipped; DVE derives the 0.01 copy (x1/99).
  4. MLP tail, two column-chunks (all-but-last-block, then the last 6):
     y = relu(o@W1+b1) @ 0.99W2 + o @ 0.01(W1@W2) + b2', with
     b2' = 0.01*(b1@W2) + b2 folded on the host. Only Relu/Identity are
     used (no act-table switches) and the linear stream reads o^T directly
     (associativity), staying off the critical path. Output is written as
     [F, Jp] fp32 and untransposed on the host.

Sharding: core c = 2*b + h handles batch b and half of b's kept destination
nodes (interleaved for balance). Sum over source axis i stays local; no
collectives.
"""
import numpy as np
import ml_dtypes

import concourse.bacc as bacc
import concourse.mybir as mybir
import concourse.tile as tile
from concourse.bass_utils import run_bass_kernel_spmd

B, N, F, H = 4, 512, 64, 128
NEG_SLOPE = 0.01
P = 128          # partitions / i-block size
NI = N // P      # number of i blocks (4)
JG = 12          # padding granularity for the kept-j count
JBW = 24         # main j-block width
N_CORES = 8

F32 = mybir.dt.float32
BF16 = mybir.dt.bfloat16
NPBF16 = np.dtype(ml_dtypes.bfloat16)

_PROG_CACHE = {}


N_PLAIN = 1    # leading blocks DMA'd plain over HWDGE (no prefill dep)


def _widths(Jp):
    """j-block widths. Block 0 is 24-wide and DMA'd plain over HWDGE with
    an explicit x-add (no prefill dependency, so the edge stream starts
    ~2.6us in; its x-add runs while ScalarE would idle anyway); widths
    taper 24 -> 12 -> 6 so the exposed relu after each late DMA shrinks,
    the last two blocks 6-wide (tail after the final DMA pays the +900ns
    DMA completion-semaphore latency)."""
    assert Jp % JG == 0
    if Jp < 5 * JG:   # tiny problems: simple 12-wide blocks
        return [JG] * (Jp // JG)
    rem = Jp - JBW - JG  # minus the 24 lead and the 6+6 tail
    a, r = divmod(rem, JBW)
    if r == 0:
        a, b = a - 1, 2
    else:
        b = r // JG
    return [JBW] + [JBW] * a + [JG] * b + [JG // 2, JG // 2]


def _split2(JW, gi, NB):
    """jw-split of the relu between ScalarE (0.833ns/elem) and DVE
    (tensor_tensor max vs zero, 2x mode: 0.52ns/elem, plus prefill duty).
    Early blocks lean on ScalarE (it idles waiting for data anyway); late
    blocks lean on DVE, whose prefill duty is over by then."""
    late = gi >= NB - 5
    if JW >= 24:
        return (10, 14) if late else (14, 10)
    if JW >= 12:
        return (3, 9) if late else (4, 8)
    return 2, JW - 2





def _build(Jp: int, z_bufs=5, u_bufs=3):
    G = len(_widths(Jp))
    nc = bacc.Bacc("TRN2", target_bir_lowering=False)

    edge_d = nc.dram_tensor("edge", [N, Jp, F], BF16, kind="ExternalInput")
    # two prescaled adjacency copies: s=0 -> 0.01*adj (raw-z stream),
    # s=1 -> 0.99*adj (relu stream); lrelu(z) = 0.01*z + 0.99*relu(z)
    adj_d = nc.dram_tensor("adj", [P, 2 * NI * Jp], BF16, kind="ExternalInput")
    # bf16 consts, one DMA: x (cols 0:NI*F, needed by the first prefill at
    # ~3us) ++ W1 (rows 0:64, H cols) ++ 0.99*W2 (rows 0:H, F cols) ++
    # 0.01*(W1@W2) (rows 0:64, F cols) ++ xkT (rows 0:64, Jp cols). The
    # linear branch of the MLP's leaky-relu splits off as o @ (0.01*W1@W2)
    # by associativity, so it reads o^T directly instead of a PSUM copy of
    # o@W1 -- off the critical path.
    XW = NI * F
    CWB = XW + H + 2 * F + Jp
    cstb_d = nc.dram_tensor("cstb", [P, CWB], BF16, kind="ExternalInput")
    # f32 consts: b1 (col 0), b2 (col 1)
    cstf_d = nc.dram_tensor("cstf", [P, 2], F32, kind="ExternalInput")
    out_d = nc.dram_tensor("out", [F, Jp], F32, kind="ExternalOutput")

    with tile.TileContext(nc) as tc:
        with tc.tile_pool(name="cpool", bufs=1) as cpool:
            # x loads first (tiny) so the first z-prefill can start ASAP
            # the bf16 const pack (incl x) loads first: the first SWDGE
            # block's prefill needs x at ~3.3us
            cb_t = cpool.tile([P, CWB], BF16)
            nc.sync.dma_start(out=cb_t[:, :], in_=cstb_d[:, :])
            adj_t = cpool.tile([P, 2 * NI * Jp], BF16)
            cf_t = cpool.tile([P, 2], F32)
            zero_t = cpool.tile([P, F], BF16)
            nc.gpsimd.memset(zero_t[:, :], 0.0)

            def load_consts():
                # only the 0.99 copy ships; the 0.01 copy is derived on DVE
                nc.sync.dma_start(
                    out=adj_t[:, NI * Jp:2 * NI * Jp],
                    in_=adj_d[:, NI * Jp:2 * NI * Jp])
                nc.sync.dma_start(out=cf_t[:, :], in_=cstf_d[:, :])

            def scale_adj():
                nc.vector.tensor_scalar(
                    out=adj_t[:, 0:NI * Jp],
                    in0=adj_t[:, NI * Jp:2 * NI * Jp],
                    scalar1=1.0 / 99.0, scalar2=None,
                    op0=mybir.AluOpType.mult)

            x_v = cb_t[:, 0:XW].rearrange("p (ib f) -> p ib f", ib=NI)
            adj_v = adj_t[:, :].rearrange("p (s ib j) -> p s ib j",
                                          s=2, ib=NI)
            w1_t = cb_t[:F, XW:XW + H]
            w2a_t = cb_t[:H, XW + H:XW + H + F]           # 0.99 * W2
            wlin_t = cb_t[:F, XW + H + F:XW + H + 2 * F]  # 0.01*(W1@W2)
            xkT_t = cb_t[:F, XW + H + 2 * F:XW + H + 2 * F + Jp]
            b1_t = cf_t[:H, 0:1]
            b2_t = cf_t[:F, 1:2]  # = 0.01*(b1 @ W2) + b2 (host-folded)

            with tc.tile_pool(name="spool", bufs=2) as spool, \
                 tc.tile_pool(name="ppool", bufs=1, space="PSUM") as ppool:
                # single PSUM accumulator for the whole aggregation, and the
                # SBUF o^T staging tile its columns drain into per block
                oT_p = ppool.tile([F, Jp], F32, name="oT")
                oTs_t = cpool.tile([F, Jp], BF16)

                widths = _widths(Jp)
                starts = [sum(widths[:i]) for i in range(len(widths))]
                NB = len(widths)

                def issue_load(gi):
                    g0, JW = starts[gi], widths[gi]
                    FW = NI * JW * F
                    x_b = x_v[:, :, None, :].broadcast_to([P, NI, JW, F])
                    z_t = spool.tile([P, FW], BF16, tag="z", bufs=z_bufs,
                                     padded_shape=[P, NI * JBW * F])
                    z4 = z_t[:, :].rearrange("p (ib jw f) -> p ib jw f",
                                             ib=NI, jw=JW)
                    e_src = edge_d[:, g0:g0 + JW, :].rearrange(
                        "(ib p) j f -> p ib j f", p=P)
                    if gi < N_PLAIN:
                        # plain HWDGE load (starts immediately, no prefill
                        # dependency); x is broadcast-added in process().
                        e_t = spool.tile([P, FW], BF16, tag="epl",
                                         bufs=N_PLAIN,
                                         padded_shape=[P, NI * JBW * F])
                        e4 = e_t[:, :].rearrange(
                            "p (ib jw f) -> p ib jw f", ib=NI, jw=JW)
                        nc.sync.dma_start(out=e4, in_=e_src)
                        if gi == N_PLAIN - 1:
                            load_consts()
                        return (z4, e4, x_b)
                    # prefill with broadcast x (DVE 4x-mode copy), then the
                    # edge slab for all 4 i-blocks lands on top via the DMA
                    # engines' inline CCE adder: z = x + e.
                    nc.vector.tensor_copy(z4, x_b)
                    nc.gpsimd.dma_start(out=z4, in_=e_src,
                                        accum_op=mybir.AluOpType.add)
                    return (z4, None, None)

                def process_relu(gi, load):
                    g0, JW = starts[gi], widths[gi]
                    FW = NI * JW * F
                    z4, e4, x_b = load
                    if e4 is not None:
                        nc.vector.tensor_tensor(out=z4, in0=e4, in1=x_b,
                                                op=mybir.AluOpType.add)

                    # u = relu(z); the 0.01*z part of lrelu rides the raw-z
                    # matmul stream against the 0.01-scaled adjacency.
                    u_t = spool.tile([P, FW], BF16, tag="u", bufs=u_bufs,
                                     padded_shape=[P, NI * JBW * F])
                    u4 = u_t[:, :].rearrange("p (ib jw f) -> p ib jw f",
                                             ib=NI, jw=JW)
                    sA, sD = _split2(JW, gi, NB)
                    assert sA + sD == JW
                    if sA:
                        nc.scalar.activation(u4[:, :, 0:sA, :],
                                             z4[:, :, 0:sA, :],
                                             mybir.ActivationFunctionType.Relu)
                    if sD:
                        zb = zero_t[:, None, None, :].broadcast_to(
                            [P, NI, sD, F])
                        nc.vector.tensor_tensor(
                            out=u4[:, :, sA:JW, :], in0=z4[:, :, sA:JW, :],
                            in1=zb, op=mybir.AluOpType.max)
                    return z4, u4

                def process_agg(gi, z4, u4):
                    g0, JW = starts[gi], widths[gi]
                    # aggregation: oT[:, j] = sum_ib (z^T@0.01adj + u^T@0.99adj)
                    # raw-z matmuls are issued first: they only need the DMA,
                    # so they overlap the relu.
                    # all raw-z matmuls first (they only need the DMA), all
                    # relu-stream matmuls after: later columns' z-work never
                    # queues behind earlier columns' relu-gated work on the
                    # PE sequencer.
                    for jw in range(JW):
                        j = g0 + jw
                        for ib in range(NI):
                            nc.tensor.matmul(
                                oT_p[:, j:j + 1],
                                z4[:, ib, jw, :],
                                adj_v[:, 0, ib, j:j + 1],
                                start=(ib == 0), stop=False)
                    for jw in range(JW):
                        j = g0 + jw
                        for ib in range(NI):
                            nc.tensor.matmul(
                                oT_p[:, j:j + 1],
                                u4[:, ib, jw, :],
                                adj_v[:, 1, ib, j:j + 1],
                                start=False, stop=(ib == NI - 1))

                def mlp(c0, c1):
                    # PSUM drain for [c0, c1) (o = oT + x_j, bf16), then the
                    # MLP with its leaky-relu split the same way as the edge
                    # stage:
                    # y = relu(o@W1+b1) @ 0.99W2 + o @ 0.01(W1@W2) + b2'
                    # with b2' = 0.01*(b1@W2) + b2 folded on the host. The
                    # linear stream reads o^T, so it runs before relu(h) is
                    # even ready; only Relu/Identity are used -> no
                    # act-table switches.
                    CW = c1 - c0
                    nc.vector.tensor_tensor(
                        out=oTs_t[:, c0:c1], in0=oT_p[:, c0:c1],
                        in1=xkT_t[:, c0:c1], op=mybir.AluOpType.add)
                    h_p = ppool.tile([H, CW], F32, tag="hp", bufs=2,
                                     padded_shape=[H, Jp])
                    nc.tensor.matmul(h_p[:, :], w1_t, oTs_t[:, c0:c1],
                                     start=True, stop=True)
                    y_p = ppool.tile([F, CW], F32, tag="yp", bufs=2,
                                     padded_shape=[F, Jp])
                    nc.tensor.matmul(y_p[:, :], wlin_t, oTs_t[:, c0:c1],
                                     start=True, stop=False)
                    h_s = spool.tile([H, CW], BF16, tag="hs",
                                     padded_shape=[H, Jp])
                    nc.scalar.activation(h_s[:, :], h_p[:, :],
                                         mybir.ActivationFunctionType.Relu,
                                         bias=b1_t)
                    nc.tensor.matmul(y_p[:, :], w2a_t, h_s[:, :],
                                     start=False, stop=True)
                    y_s = spool.tile([F, CW], F32, tag="ys",
                                     padded_shape=[F, Jp])
                    if c1 == Jp:
                        # final chunk: DVE is idle here while ACT's sequencer
                        # still drains earlier updates
                        nc.vector.tensor_tensor(
                            out=y_s[:, :], in0=y_p[:, :],
                            in1=b2_t.broadcast_to([F, CW]),
                            op=mybir.AluOpType.add)
                    else:
                        nc.scalar.activation(
                            y_s[:, :], y_p[:, :],
                            mybir.ActivationFunctionType.Identity,
                            bias=b2_t)
                    nc.sync.dma_start(out=out_d[:, c0:c1], in_=y_s[:, :])

                # software pipeline: loads run LOOKAHEAD blocks ahead of
                # processing so every engine's program order interleaves
                # next-block prefill/DMA before current-block consumers.
                # The MLP runs just twice: everything up to the last two
                # blocks as soon as their aggregation is done (hidden under
                # the stream), the last two blocks' columns at the end.
                # the MLP/output run in two chunks: everything but the last
                # block's columns right after the second-to-last block's
                # aggregation (hidden under the final DMA + its relu), the
                # last block's columns at the very end.
                LOOKAHEAD = 3
                cut = starts[NB - 1]

                def process(gi, load):
                    zu = process_relu(gi, load)
                    if gi == 0:
                        scale_adj()
                    process_agg(gi, *zu)

                pend = []
                for gi in range(NB):
                    pend.append((gi, issue_load(gi)))
                    if len(pend) > LOOKAHEAD:
                        process(*pend.pop(0))
                # drain: the last block's relu is issued BEFORE MLP-A's
                # PSUM drain so it isn't queued behind it on DVE; its
                # aggregation (PE) runs while MLP-A drains.
                for item in pend[:-1]:
                    process(*item)
                gi_last, load_last = pend[-1]
                with tc.high_priority():
                    zu = process_relu(gi_last, load_last)
                if cut:
                    mlp(0, cut)
                with tc.high_priority():
                    process_agg(gi_last, *zu)
                    mlp(cut, Jp)

    nc.compile()
    return nc


def _get_prog(Jp: int):
    if Jp not in _PROG_CACHE:
        _PROG_CACHE[Jp] = _build(Jp)
    return _PROG_CACHE[Jp]


def kernel(x, adj, edge_attr, mask, W1, b1, W2, b2):
    x = np.ascontiguousarray(np.asarray(x, dtype=np.float32))
    adj = np.ascontiguousarray(np.asarray(adj, dtype=np.float32))
    edge_attr = np.ascontiguousarray(np.asarray(edge_attr, dtype=np.float32))
    mask = np.asarray(mask)
    W1 = np.asarray(W1, dtype=np.float32)
    b1 = np.asarray(b1, dtype=np.float32)
    W2 = np.asarray(W2, dtype=np.float32)
    b2 = np.asarray(b2, dtype=np.float32)

    # core c = 2*b + h: batch b, interleaved half h of b's kept nodes
    core_jj = []
    for b in range(B):
        jj = np.flatnonzero(mask[b])
        core_jj.append(jj[0::2])
        core_jj.append(jj[1::2])
    maxJ = max((len(jj) for jj in core_jj), default=1)
    Jp = max(JG, ((maxJ + JG - 1) // JG) * JG)

    nc = _get_prog(Jp)

    XW = NI * F
    CWB = XW + H + 2 * F + Jp
    in_maps = []
    for c, jj in enumerate(core_jj):
        b = c // 2
        J = len(jj)
        edge_c = np.zeros((N, Jp, F), NPBF16)
        if J:
            edge_c[:, :J] = edge_attr[b][:, jj, :].astype(NPBF16)
        adj_c = np.zeros((N, Jp), np.float32)
        if J:
            adj_c[:, :J] = adj[b][:, jj]
        adj_ibpj = adj_c.reshape(NI, P, Jp).transpose(1, 0, 2)  # [P, NI, Jp]
        adj_r = np.concatenate(
            [NEG_SLOPE * adj_ibpj, (1.0 - NEG_SLOPE) * adj_ibpj],
            axis=1).reshape(P, 2 * NI * Jp).astype(NPBF16)
        x_r = x[b].reshape(NI, P, F).transpose(1, 0, 2).reshape(
            P, NI * F).astype(NPBF16)
        cstb = np.zeros((P, CWB), NPBF16)
        cstb[:, 0:XW] = x_r
        cstb[:F, XW:XW + H] = W1.astype(NPBF16)
        cstb[:H, XW + H:XW + H + F] = ((1.0 - NEG_SLOPE) * W2).astype(NPBF16)
        cstb[:F, XW + H + F:XW + H + 2 * F] = (
            NEG_SLOPE * (W1 @ W2)).astype(NPBF16)
        if J:
            cstb[:F, XW + H + 2 * F:XW + H + 2 * F + J] = (
                x[b][jj].T.astype(NPBF16))
        cstf = np.zeros((P, 2), np.float32)
        cstf[:H, 0] = b1
        cstf[:F, 1] = NEG_SLOPE * (b1 @ W2) + b2
        in_maps.append({
            "edge": edge_c, "adj": np.ascontiguousarray(adj_r),
            "cstb": cstb, "cstf": cstf,
        })

    res = run_bass_kernel_spmd(nc, in_maps, list(range(N_CORES)))

    out = np.zeros((B, N, F), np.float32)
    for c, jj in enumerate(core_jj):
        b = c // 2
        if len(jj):
            out[b][jj] = res.results[c]["out"][:, :len(jj)].T
    return out
